# revision 4
# baseline (speedup 1.0000x reference)
"""Deformable-conv kernel for 8 trn2 NeuronCores.

The module samples x at only K*K=3x3 points (grid is [B,3,3,2], identical
coords across the batch), so `shifted` is [B,C,3,3] and the conv output is
[B,CO,3,3].  Host does the 36-point gather + bilinear + im2col (tiny);
the 8 cores run the conv as a contraction-sharded matmul:

    out_rows[row, co] = sum_k patch[row, k] * wmat[k, co],
    k = (c, kh, kw) in [0, 2304), rows = (b, oh, ow) in [0, 288)

Core i takes k-slice [288*i, 288*(i+1)) (= 32 input channels), computes a
partial [CO, 288] on the PE array, host reduces the 8 partials + bias.
"""

import numpy as np

import concourse.bacc as bacc
import concourse.bass as bass
import concourse.mybir as mybir
import concourse.tile as tile
from concourse.bass_utils import run_bass_kernel_spmd

B, C, H, W = 32, 256, 224, 224
K = 3
CO = 256
N_CORES = 8
KTOT = C * K * K            # 2304 contraction size
KSH = KTOT // N_CORES       # 288 contraction rows per core
ROWS = B * K * K            # 288 output rows (b, oh, ow)

TRACE = False               # test harness may flip this
LAST_RESULT = None          # BassKernelResults of the last run

_nc_cache = None


def _build_nc():
    nc = bacc.Bacc("TRN2", target_bir_lowering=False, debug=False)
    p_t = nc.dram_tensor("p_t", [KSH, ROWS], mybir.dt.float32, kind="ExternalInput")
    w_k = nc.dram_tensor("w_k", [KSH, CO], mybir.dt.float32, kind="ExternalInput")
    out_p = nc.dram_tensor("out_p", [CO, ROWS], mybir.dt.float32, kind="ExternalOutput")

    k_tiles = [(0, 128), (128, 128), (256, 32)]
    m_tiles = [(0, 128), (128, 128)]

    with tile.TileContext(nc) as tc:
        with (
            tc.tile_pool(name="sb", bufs=1) as sb,
            tc.tile_pool(name="ps", bufs=2, space="PSUM") as ps,
            tc.tile_pool(name="ob", bufs=2) as ob,
        ):
            pt_tiles = []
            wk_tiles = []
            for k0, kn in k_tiles:
                t = sb.tile([kn, ROWS], mybir.dt.float32, tag=f"pt{k0}")
                nc.sync.dma_start(out=t, in_=p_t[k0 : k0 + kn, :])
                pt_tiles.append(t)
                w = sb.tile([kn, CO], mybir.dt.float32, tag=f"wk{k0}")
                nc.sync.dma_start(out=w, in_=w_k[k0 : k0 + kn, :])
                wk_tiles.append(w)
            for m0, mn in m_tiles:
                acc = ps.tile([mn, ROWS], mybir.dt.float32)
                for ki, (k0, kn) in enumerate(k_tiles):
                    nc.tensor.matmul(
                        acc,
                        wk_tiles[ki][:, m0 : m0 + mn],
                        pt_tiles[ki],
                        start=(ki == 0),
                        stop=(ki == len(k_tiles) - 1),
                    )
                o = ob.tile([mn, ROWS], mybir.dt.float32)
                nc.vector.tensor_copy(o, acc)
                nc.sync.dma_start(out=out_p[m0 : m0 + mn, :], in_=o)
    nc.finalize()
    return nc


def _get_nc():
    global _nc_cache
    if _nc_cache is None:
        _nc_cache = _build_nc()
    return _nc_cache


def _host_sample(x, offsets):
    """Mirror of the reference grid computation + bilinear gather (f32)."""
    f32 = np.float32
    ii, jj = np.meshgrid(np.arange(K, dtype=f32), np.arange(K, dtype=f32), indexing="ij")
    gx = (ii + offsets[..., 0]) / f32(H - 1)
    gy = (jj + offsets[..., 1]) / f32(H - 1)
    ix = ((gx + f32(1.0)) * f32(W) - f32(1.0)) * f32(0.5)
    iy = ((gy + f32(1.0)) * f32(H) - f32(1.0)) * f32(0.5)
    x0 = np.floor(ix)
    y0 = np.floor(iy)
    wx1 = ix - x0
    wx0 = f32(1.0) - wx1
    wy1 = iy - y0
    wy0 = f32(1.0) - wy1

    shifted = None
    corners = [
        (x0, y0, wx0 * wy0),
        (x0 + f32(1.0), y0, wx1 * wy0),
        (x0, y0 + f32(1.0), wx0 * wy1),
        (x0 + f32(1.0), y0 + f32(1.0), wx1 * wy1),
    ]
    for xi, yi, wgt in corners:
        xii = xi.astype(np.int32)
        yii = yi.astype(np.int32)
        valid = (xii >= 0) & (xii < W) & (yii >= 0) & (yii < H)
        xc = np.clip(xii, 0, W - 1)
        yc = np.clip(yii, 0, H - 1)
        v = x[:, :, yc, xc]  # [B, C, 3, 3]
        term = v * (wgt * valid.astype(f32))
        shifted = term if shifted is None else shifted + term
    return shifted  # [B, C, 3, 3]


def _im2col_t(shifted):
    """patchT[(c,kh,kw), (b,oh,ow)] for the pad=1 stride=1 3x3 conv."""
    sp = np.zeros((B, C, K + 2, K + 2), np.float32)
    sp[:, :, 1 : K + 1, 1 : K + 1] = shifted
    win = np.lib.stride_tricks.sliding_window_view(sp, (K, K), axis=(2, 3))
    # win: [b, c, oh, ow, kh, kw]
    return win.transpose(1, 4, 5, 0, 2, 3).reshape(KTOT, ROWS)


def kernel(**inputs):
    global LAST_RESULT
    x = np.asarray(inputs["x"], dtype=np.float32)
    offsets = np.asarray(inputs["offsets"], dtype=np.float32)
    conv_w = np.asarray(inputs["conv_w"], dtype=np.float32)
    conv_b = np.asarray(inputs["conv_b"], dtype=np.float32)

    shifted = _host_sample(x, offsets)
    patch_t = _im2col_t(shifted)
    wmat = conv_w.transpose(1, 2, 3, 0).reshape(KTOT, CO)

    in_maps = []
    for i in range(N_CORES):
        sl = slice(i * KSH, (i + 1) * KSH)
        in_maps.append(
            {
                "p_t": np.ascontiguousarray(patch_t[sl]),
                "w_k": np.ascontiguousarray(wmat[sl]),
            }
        )

    res = run_bass_kernel_spmd(
        _get_nc(), in_maps, core_ids=list(range(N_CORES)), trace=TRACE
    )
    LAST_RESULT = res

    acc = res.results[0]["out_p"].astype(np.float32, copy=True)
    for r in res.results[1:]:
        acc += r["out_p"]
    acc += conv_b[:, None]
    return np.ascontiguousarray(acc.reshape(CO, B, K, K).transpose(1, 0, 2, 3))


# revision 7
# speedup vs baseline: 1.0582x; 1.0582x over previous
"""Deformable-conv kernel for 8 trn2 NeuronCores.

The module samples x at only K*K=3x3 points (grid is [B,3,3,2], identical
coords across the batch), so `shifted` is [B,C,3,3] and the conv output is
[B,CO,3,3].  Host does the 36-point gather + bilinear + im2col (tiny);
the 8 cores run the conv as a contraction-sharded matmul:

    out_rows[row, co] = sum_k patch[row, k] * wmat[k, co],
    k = (c, kh, kw) in [0, 2304), rows = (b, oh, ow) in [0, 288)

Core i takes k-slice [288*i, 288*(i+1)) (= 32 input channels), computes a
partial [CO, 288] on the PE array, host reduces the 8 partials + bias.
"""

import numpy as np

import concourse.bacc as bacc
import concourse.bass as bass
import concourse.mybir as mybir
import concourse.tile as tile
from concourse.bass_utils import run_bass_kernel_spmd

B, C, H, W = 32, 256, 224, 224
K = 3
CO = 256
N_CORES = 8
KTOT = C * K * K            # 2304 contraction size
KSH = KTOT // N_CORES       # 288 contraction rows per core
ROWS = B * K * K            # 288 output rows (b, oh, ow)

TRACE = False               # test harness may flip this
LAST_RESULT = None          # BassKernelResults of the last run

_nc_cache = None


K_TILES = [(0, 128), (128, 128), (256, 32)]
M_TILES = [(0, 128), (128, 128)]


def _build_nc():
    """Raw bacc kernel: explicit per-engine streams, no Tile tail barrier.

    SP queue DMAs the patch k-tiles, Activation queue DMAs the weight
    k-tiles (two HWDGE rings in parallel); PE runs the 6 accumulating
    fp32 matmuls gated per k-tile; DVE copies PSUM->SBUF; SP DMAs out.
    """
    f32 = mybir.dt.float32
    nc = bacc.Bacc("TRN2", target_bir_lowering=False, debug=False)
    p_t = nc.dram_tensor("p_t", [KSH, ROWS], f32, kind="ExternalInput")
    w_k = nc.dram_tensor("w_k", [KSH, CO], f32, kind="ExternalInput")
    out_p = nc.dram_tensor("out_p", [CO, ROWS], f32, kind="ExternalOutput")

    with (
        nc.sbuf_tensor("pt0", [128, ROWS], f32) as pt0,
        nc.sbuf_tensor("pt1", [128, ROWS], f32) as pt1,
        nc.sbuf_tensor("pt2", [32, ROWS], f32) as pt2,
        nc.sbuf_tensor("wk0", [128, CO], f32) as wk0,
        nc.sbuf_tensor("wk1", [128, CO], f32) as wk1,
        nc.sbuf_tensor("wk2", [32, CO], f32) as wk2,
        nc.sbuf_tensor("ob0", [128, ROWS], f32) as ob0,
        nc.sbuf_tensor("ob1", [128, ROWS], f32) as ob1,
        nc.psum_tensor("ps0", [128, ROWS], f32) as ps0,
        nc.psum_tensor("ps1", [128, ROWS], f32) as ps1,
        nc.semaphore("sem_k0") as sem_k0,
        nc.semaphore("sem_k1") as sem_k1,
        nc.semaphore("sem_k2") as sem_k2,
        nc.semaphore("sem_mm") as sem_mm,
        nc.semaphore("sem_cp") as sem_cp,
        nc.semaphore("sem_out") as sem_out,
        nc.Block() as block,
    ):
        pt = [pt0, pt1, pt2]
        wk = [wk0, wk1, wk2]
        ps = [ps0, ps1]
        ob = [ob0, ob1]
        sem_k = [sem_k0, sem_k1, sem_k2]

        @block.sync
        def _(sync):
            for ki, (k0, kn) in enumerate(K_TILES):
                sync.dma_start(pt[ki][:], p_t[k0 : k0 + kn, :]).then_inc(sem_k[ki], 16)
            for mi in range(2):
                sync.wait_ge(sem_cp, mi + 1)
                sync.dma_start(
                    out_p[mi * 128 : (mi + 1) * 128, :], ob[mi][:]
                ).then_inc(sem_out, 16)
            sync.wait_ge(sem_out, 32)

        @block.scalar
        def _(scalar):
            for ki, (k0, kn) in enumerate(K_TILES):
                scalar.dma_start(wk[ki][:], w_k[k0 : k0 + kn, :]).then_inc(sem_k[ki], 16)

        @block.tensor
        def _(tensor):
            last = len(K_TILES) - 1
            for ki in range(len(K_TILES)):
                tensor.wait_ge(sem_k[ki], 32)
                for mi in range(2):
                    mm = tensor.matmul(
                        ps[mi][:],
                        wk[ki][:, mi * 128 : (mi + 1) * 128],
                        pt[ki][:],
                        start=(ki == 0),
                        stop=(ki == last),
                    )
                    if ki == last:
                        mm.then_inc(sem_mm)

        @block.vector
        def _(vector):
            for mi in range(2):
                vector.wait_ge(sem_mm, mi + 1)
                vector.tensor_copy(ob[mi][:], ps[mi][:]).then_inc(sem_cp, 1)

    nc.finalize()
    return nc


def _get_nc():
    global _nc_cache
    if _nc_cache is None:
        _nc_cache = _build_nc()
    return _nc_cache


def _host_sample(x, offsets):
    """Mirror of the reference grid computation + bilinear gather (f32)."""
    f32 = np.float32
    ii, jj = np.meshgrid(np.arange(K, dtype=f32), np.arange(K, dtype=f32), indexing="ij")
    gx = (ii + offsets[..., 0]) / f32(H - 1)
    gy = (jj + offsets[..., 1]) / f32(H - 1)
    ix = ((gx + f32(1.0)) * f32(W) - f32(1.0)) * f32(0.5)
    iy = ((gy + f32(1.0)) * f32(H) - f32(1.0)) * f32(0.5)
    x0 = np.floor(ix)
    y0 = np.floor(iy)
    wx1 = ix - x0
    wx0 = f32(1.0) - wx1
    wy1 = iy - y0
    wy0 = f32(1.0) - wy1

    shifted = None
    corners = [
        (x0, y0, wx0 * wy0),
        (x0 + f32(1.0), y0, wx1 * wy0),
        (x0, y0 + f32(1.0), wx0 * wy1),
        (x0 + f32(1.0), y0 + f32(1.0), wx1 * wy1),
    ]
    for xi, yi, wgt in corners:
        xii = xi.astype(np.int32)
        yii = yi.astype(np.int32)
        valid = (xii >= 0) & (xii < W) & (yii >= 0) & (yii < H)
        xc = np.clip(xii, 0, W - 1)
        yc = np.clip(yii, 0, H - 1)
        v = x[:, :, yc, xc]  # [B, C, 3, 3]
        term = v * (wgt * valid.astype(f32))
        shifted = term if shifted is None else shifted + term
    return shifted  # [B, C, 3, 3]


def _im2col_t(shifted):
    """patchT[(c,kh,kw), (b,oh,ow)] for the pad=1 stride=1 3x3 conv."""
    sp = np.zeros((B, C, K + 2, K + 2), np.float32)
    sp[:, :, 1 : K + 1, 1 : K + 1] = shifted
    win = np.lib.stride_tricks.sliding_window_view(sp, (K, K), axis=(2, 3))
    # win: [b, c, oh, ow, kh, kw]
    return win.transpose(1, 4, 5, 0, 2, 3).reshape(KTOT, ROWS)


def kernel(**inputs):
    global LAST_RESULT
    x = np.asarray(inputs["x"], dtype=np.float32)
    offsets = np.asarray(inputs["offsets"], dtype=np.float32)
    conv_w = np.asarray(inputs["conv_w"], dtype=np.float32)
    conv_b = np.asarray(inputs["conv_b"], dtype=np.float32)

    shifted = _host_sample(x, offsets)
    patch_t = _im2col_t(shifted)
    wmat = conv_w.transpose(1, 2, 3, 0).reshape(KTOT, CO)

    in_maps = []
    for i in range(N_CORES):
        sl = slice(i * KSH, (i + 1) * KSH)
        in_maps.append(
            {
                "p_t": np.ascontiguousarray(patch_t[sl]),
                "w_k": np.ascontiguousarray(wmat[sl]),
            }
        )

    res = run_bass_kernel_spmd(
        _get_nc(), in_maps, core_ids=list(range(N_CORES)), trace=TRACE
    )
    LAST_RESULT = res

    acc = res.results[0]["out_p"].astype(np.float32, copy=True)
    for r in res.results[1:]:
        acc += r["out_p"]
    acc += conv_b[:, None]
    return np.ascontiguousarray(acc.reshape(CO, B, K, K).transpose(1, 0, 2, 3))
